# revision 16
# baseline (speedup 1.0000x reference)
"""Trainium2 Bass kernel for LogicGatedSNN.

Math:
  w = ternarize(synapse_states)            # {-1,0,1}, exact in fp8
  current = spike_input @ w.T              # fp8 matmul, fp32 PSUM accum
  spikes[b,o] = (current[b,o] - T[o] >= 0)
where T[o] folds threshold/membrane/refractory:
  non-refractory: T = thr - DECAY*vmem
  refractory:     T = +-1e30 depending on sign of (DECAY*vmem - thr)

The kernel computes psum[o, b] = -current (the ternarize emits NEGATED
weights so it fits one fused scalar_tensor_tensor op), so the epilogue
is a single per-partition-scalar compare: spike = (psum <= -T[o]).

Sharding: 8 cores = 2 (batch) x 4 (out_features). Per core:
  spikeT shard [4096, 2048] fp8, synT shard [4096, 1024] fp32.
Host prep: spike is exactly {0,1} so the fp8 cast is lossless; both
operands are uploaded pre-transposed (K=IN on rows) so the kernel needs
no on-chip data transposes. Output is produced [o, b] per core and
transposed back on host.

Per-core dataflow:
  - synT: fp32 DMA in (4 m-chunks per DMA) -> 2-op ternarize
    (ta = is_gt(x,1) on Pool; W' = (x is_lt -1) - ta fused on DVE,
    8 per-o-chunk writes to satisfy the 3-canonical-dim AP limit)
    -> negated ternary, DR-packed W'[c][128, 2, 8, 2, 128] fp8
  - spikeT: fp8 DMA in -> S[c][128, 4, 2048]
  - matmul: fp8 DoubleRow (K=256/instr, 157 TF/s). The DR stationary
    MUST be one contiguous 256B run per partition: with a strided
    [128, 2, 128] stationary AP, DR Ldweights measured ~30x slower on
    real HW (~+20ms launch wall), so W' is stored packed per (m-pair,
    o-chunk). psum [128o, 512b] accumulates 16 m-pairs; each stationary
    is reused across 4 moving-S matmuls
  - negT: computed in [8, 128] layout, moved to per-partition [128, 8]
    via the 2-byte hardware transpose unit (bf16 round-trip; exact for
    these magnitudes) -- a rearranged 4B-stride DMA measured ~10ms of
    launch overhead, so it is deliberately NOT used
  - epilogue: is_le(psum, negT[o]) (per-partition scalar) -> fp8 out
"""

import sys

if "/opt/trn_rl_repo" not in sys.path:
    sys.path.insert(0, "/opt/trn_rl_repo")

import numpy as np

B, IN, OUT = 4096, 4096, 4096
GB, GO = 2, 4  # core grid: batch x out_features
DECAY = 0.8
_TENSORS = {}


def build_core_program(nc, tc, bs, os_, in_, instance=0):
    """Emit the per-core program. bs/os_/in_ = per-core shard dims."""
    import concourse.mybir as mybir
    from concourse.bass import ts

    FP32 = mybir.dt.float32
    FP8 = mybir.dt.float8e4
    Op = mybir.AluOpType
    DR = mybir.MatmulPerfMode.DoubleRow

    if instance == 0:
        spt = nc.dram_tensor("spt", [in_, bs], FP8, kind="ExternalInput")
        synt = nc.dram_tensor("synt", [in_, os_], FP32, kind="ExternalInput")
        thr = nc.dram_tensor("thr", [1, os_], FP32, kind="ExternalInput")
        vmem = nc.dram_tensor("vmem", [1, os_], FP32, kind="ExternalInput")
        refrac = nc.dram_tensor("refrac", [1, os_], FP32, kind="ExternalInput")
        # spikes are 0/1 -> exact in fp8; host casts back to fp32
        out = nc.dram_tensor("spikes", [os_, bs], FP8, kind="ExternalOutput")
        _TENSORS.update(
            spt=spt, synt=synt, thr=thr, vmem=vmem, refrac=refrac, out=out
        )
    else:
        spt, synt, thr, vmem, refrac, out = (
            _TENSORS["spt"],
            _TENSORS["synt"],
            _TENSORS["thr"],
            _TENSORS["vmem"],
            _TENSORS["refrac"],
            _TENSORS["out"],
        )

    NC_ = os_ // 128  # o-chunks (psum partition tiles)
    MP = in_ // 256  # m-pair count (matmul k-chunks = 2*MP)
    NBB = bs // 512  # moving-dim tiles
    SC = in_ // 512  # spike DMA chunks (4 m-chunks each)
    WC = in_ // 512  # weight chunks (4 m-chunks each)

    with (
        tc.tile_pool(name="spool", bufs=1) as spool,
        tc.tile_pool(name="wpool", bufs=1) as wpool,
        tc.tile_pool(name="synpool", bufs=3) as synpool,
        tc.tile_pool(name="tpool", bufs=3) as tpool,
        tc.tile_pool(name="outpool", bufs=4) as outpool,
        tc.tile_pool(name="miscpool", bufs=1) as miscpool,
        tc.tile_pool(name="pspool", bufs=8, space="PSUM") as pspool,
    ):
        # ---- negT[o] = -(effective threshold), computed in [8, 128] ----
        tv = miscpool.tile([8, 128], FP32, tag="tv")
        vv = miscpool.tile([8, 128], FP32, tag="vv")
        rv = miscpool.tile([8, 128], FP32, tag="rv")
        nc.sync.dma_start(tv[:], thr[:, :].rearrange("a (c p) -> (a c) p", p=128))
        nc.sync.dma_start(vv[:], vmem[:, :].rearrange("a (c p) -> (a c) p", p=128))
        nc.sync.dma_start(rv[:], refrac[:, :].rearrange("a (c p) -> (a c) p", p=128))
        c0 = miscpool.tile([8, 128], FP32, tag="c0")
        nc.vector.tensor_scalar(c0[:], vv[:], DECAY, None, Op.mult)
        nc.vector.tensor_tensor(c0[:], c0[:], tv[:], Op.subtract)  # decay*v - thr
        big = miscpool.tile([8, 128], FP32, tag="big")
        nc.vector.tensor_scalar(big[:], c0[:], 0.0, None, Op.is_ge)
        nc.vector.tensor_scalar(big[:], big[:], 2e30, -1e30, Op.mult, Op.add)
        r01 = miscpool.tile([8, 128], FP32, tag="r01")
        nc.vector.tensor_scalar(r01[:], rv[:], 0.0, None, Op.is_gt)
        # negT = c0 + r01 * (big - c0)
        nc.vector.tensor_tensor(big[:], big[:], c0[:], Op.subtract)
        nc.vector.tensor_tensor(big[:], big[:], r01[:], Op.mult)
        negT = miscpool.tile([8, 128], FP32, tag="negT")
        nc.vector.tensor_tensor(negT[:], c0[:], big[:], Op.add)
        # per-partition layout negT_t[p, c] = negT[o = c*128 + p], via the
        # hardware 2-byte transpose unit (bf16: 0.5 thr exact, 1e30 in range)
        BF16 = mybir.dt.bfloat16
        ntb = miscpool.tile([32, 128], BF16, tag="ntb")
        nc.vector.memset(ntb[:], 0.0)
        nc.vector.tensor_scalar(ntb[0:8, :], negT[:], 0.0, None, Op.add)
        ntt = miscpool.tile([128, 32], BF16, tag="ntt")
        nc.sync.dma_start(ntt[:], ntb[:], transpose=True)
        negT_t = miscpool.tile([128, 8], FP32, tag="negT_t")
        nc.vector.tensor_scalar(negT_t[:], ntt[:, 0:8], 0.0, None, Op.add)

        # ---- interleaved loads: syn chunks feed the ternarize pipeline in
        # consumption order; spike chunks trickle in between on their own
        # queue. W' = negated ternary, 2 ops per chunk, split DVE/Pool ----
        Ss = [
            spool.tile([128, 4, bs], FP8, tag=f"S{c}", name=f"S{c}")
            for c in range(SC)
        ]
        # W2[c][p, mp_in, oc, j, o'] = w'[k=(2*(2c+mp_in)+j)*128+p, oc*128+o']
        # -> each DoubleRow stationary slice is one contiguous 256B run per
        # partition (strided DR stationaries measured ~30x slower Ldweights)
        Ws = [
            wpool.tile([128, 2, os_ // 128, 2, 128], FP8, tag=f"W{c}", name=f"W{c}")
            for c in range(WC)
        ]
        for c in range(WC):
            st = synpool.tile([128, 4, os_], FP32, tag="st", name="st")
            nc.sync.dma_start(
                st[:], synt[ts(c, 512), :].rearrange("(m p) o -> p m o", p=128)
            )
            nc.scalar.dma_start(
                Ss[c][:], spt[ts(c, 512), :].rearrange("(m p) b -> p m b", p=128)
            )
            # Pool does the compare (scalar_tensor_tensor is PE-illegal on
            # Pool), DVE the fused second compare + subtract
            ta = tpool.tile([128, 4, os_], FP8, tag="ta", name="ta")
            nc.gpsimd.tensor_scalar(ta[:], st[:], 1.0, None, Op.is_gt)
            # W' = (st < -1) - (st > 1)  == -ternarize(st), written packed.
            # One op per o-chunk keeps both APs <=3 canonical dims (walrus
            # verifier limit); iteration order (mp_in, j, o') matches on both
            # sides.
            for oc in range(os_ // 128):
                nc.vector.scalar_tensor_tensor(
                    Ws[c][:, :, oc, :, :],
                    st[:, :, ts(oc, 128)],
                    -1.0,
                    ta[:, :, ts(oc, 128)],
                    Op.is_lt,
                    Op.subtract,
                )

        # ---- main sweep: psum groups of 2 o-chunks x 4 moving tiles ----
        for og in range(NC_ // 2):
            pss = [
                [pspool.tile([128, 512], FP32, tag="ps", name="ps") for _ in range(NBB)]
                for _ in range(2)
            ]
            for mp in range(MP):
                wc, wj = divmod(mp, 2)  # W chunk, m-pair within chunk
                sc, sj = divmod(mp, 2)  # S chunk, m-pair within chunk
                for oi in range(2):
                    oc = og * 2 + oi
                    lhsT = Ws[wc][:, wj, oc, :, :]
                    for bb in range(NBB):
                        nc.tensor.matmul(
                            pss[oi][bb][:],
                            lhsT,
                            Ss[sc][:, 2 * sj : 2 * sj + 2, ts(bb, 512)],
                            start=(mp == 0),
                            stop=(mp == MP - 1),
                            perf_mode=DR,
                        )
            for oi in range(2):
                oc = og * 2 + oi
                ob = outpool.tile([128, NBB, 512], FP8, tag="ob", name="ob")
                for bb in range(NBB):
                    # psum = -current; spike = (current >= T) == (psum <= -T)
                    nc.vector.tensor_scalar(
                        ob[:, bb, :],
                        pss[oi][bb][:],
                        negT_t[:, oc : oc + 1],
                        None,
                        Op.is_le,
                    )
                nc.scalar.dma_start(out[ts(oc, 128), :], ob[:])
    return out


def make_nc(bs=B // GB, os_=OUT // GO, in_=IN, repeat=1):
    from concourse import bacc
    from concourse.tile import TileContext

    nc = bacc.Bacc(trn_type="TRN2")
    with TileContext(nc) as tc:
        for r in range(repeat):
            build_core_program(nc, tc, bs, os_, in_, instance=r)
    nc.compile()
    return nc


def make_in_maps(
    spike_input,
    synapse_states,
    membrane_potential,
    adaptive_threshold,
    refractory_count,
):
    import ml_dtypes

    FP8 = ml_dtypes.float8_e4m3

    spike_input = np.asarray(spike_input, dtype=np.float32)
    synapse_states = np.asarray(synapse_states, dtype=np.float32)
    membrane_potential = np.asarray(membrane_potential, dtype=np.float32)
    adaptive_threshold = np.asarray(adaptive_threshold, dtype=np.float32)
    refractory_count = np.asarray(refractory_count, dtype=np.float32)

    bs, os_ = B // GB, OUT // GO
    # spike values are exactly {0.0, 1.0} -> fp8 cast is lossless
    spt_all = [
        np.ascontiguousarray(spike_input[bi * bs : (bi + 1) * bs].astype(FP8).T)
        for bi in range(GB)
    ]
    synt_all = [
        np.ascontiguousarray(synapse_states[oj * os_ : (oj + 1) * os_].T)
        for oj in range(GO)
    ]
    in_maps = []
    for c in range(GB * GO):
        bi, oj = divmod(c, GO)
        in_maps.append(
            {
                "spt": spt_all[bi],
                "synt": synt_all[oj],
                "thr": adaptive_threshold[None, oj * os_ : (oj + 1) * os_],
                "vmem": membrane_potential[None, oj * os_ : (oj + 1) * os_],
                "refrac": refractory_count[None, oj * os_ : (oj + 1) * os_],
            }
        )
    return in_maps


_NC_CACHE = {}


def kernel(
    spike_input,
    synapse_states,
    membrane_potential,
    adaptive_threshold,
    refractory_count,
    _return_results=False,
):
    from concourse.bass_utils import run_bass_kernel_spmd

    bs, os_ = B // GB, OUT // GO
    if "nc" not in _NC_CACHE:
        _NC_CACHE["nc"] = make_nc(bs, os_, IN)
    nc = _NC_CACHE["nc"]

    in_maps = make_in_maps(
        spike_input,
        synapse_states,
        membrane_potential,
        adaptive_threshold,
        refractory_count,
    )

    res = run_bass_kernel_spmd(nc, in_maps, core_ids=list(range(GB * GO)))

    full = np.empty((B, OUT), dtype=np.float32)
    for c in range(GB * GO):
        bi, oj = divmod(c, GO)
        # per-core output is [o, b]; transpose back
        full[bi * bs : (bi + 1) * bs, oj * os_ : (oj + 1) * os_] = (
            res.results[c]["spikes"].T.astype(np.float32)
        )
    if _return_results:
        return full, res
    return full
